# revision 40
# baseline (speedup 1.0000x reference)
"""Trainium2 Bass kernel for quantized attention (nn_Attention_own_quan).

Full-input contract: kernel(**inputs) takes the unsharded inputs and returns
the full output. Internally shards (batch, head-group) across 8 NeuronCores:
core c handles batch c//2 and heads [4*(c%2), 4*(c%2)+4).

Engine assignment is driven by measured TRN2 behavior:
 - softmax exp paces the kernel on ScalarE (PSUM->SBUF, accum_out row sums)
 - GPSIMD runs only (MULTIPLY,ADD)-shaped tensor_scalar ops (its one fast
   microcode path, ~1ns/elem/lane); everything else would run 5-15x slower
 - probability quantize = two (MULT,ADD) passes via a scaled magic constant:
     eq = e*(8192*255/sum) + 1.5*2^23*8192   (GPSIMD; single RNE rounding)
     pq = eq*2^-13 - 1.5*2^23                (DVE 1x odd-split -> bf16, exact)
 - q/k/v quantize: DVE evacuates PSUM with (mult cs, add MAGIC) (single
   rounding), GPSIMD finishes (mult,add -MAGIC) on 2048-wide groups, and a
   cheap bf16 DVE pass applies the +-[128,127] clamp (round-then-clamp equals
   the reference's clamp-then-round on integer rails).
"""

import sys

sys.path.insert(0, "/opt/trn_rl_repo")

import numpy as np

import concourse.bacc as bacc
import concourse.mybir as mybir
import concourse.tile as tile
from concourse.bass_utils import run_bass_kernel_spmd

F32 = mybir.dt.float32
FP16 = mybir.dt.float16
BF16 = mybir.dt.bfloat16
AF = mybir.ActivationFunctionType
OP = mybir.AluOpType

B, S, D = 4, 2048, 512
H, DH = 8, 64
N_CORES = 8
HPC = H // 2          # heads per core = 4
EPC = HPC * DH        # head-dim columns per core = 256
MAGIC = float(np.float32(12582912.0))   # 1.5 * 2**23
M8 = float(np.float32(MAGIC * 8192.0))  # 1.5 * 2**36 (round to multiples of 8192)
INV8K = float(np.float32(2.0 ** -13))
C1536 = float(np.float32(1536.0 - MAGIC))
OFF = 1536.0

_prog_cache = {}


def _build(consts):
    """Build the single-core Bass/Tile program (SPMD across 8 cores)."""
    (rs0, rswq, rswk, rswv, rswo, cq, ck, cv, ce, wclamp, cx, cout) = consts

    nc = bacc.Bacc("TRN2", target_bir_lowering=False, debug=False)

    hsT = nc.declare_dram_parameter("hsT", [D, S], F32, isOutput=False)
    wqT = nc.declare_dram_parameter("wqT", [D, EPC], F32, isOutput=False)
    wkT = nc.declare_dram_parameter("wkT", [D, EPC], F32, isOutput=False)
    wvT = nc.declare_dram_parameter("wvT", [D, EPC], F32, isOutput=False)
    woT = nc.declare_dram_parameter("woT", [EPC, D], F32, isOutput=False)
    outT = nc.declare_dram_parameter("outT", [D, S], F32, isOutput=True)

    DT = D // 128      # 4 d-tiles
    ET = EPC // 128    # 2 e-tiles
    ST = S // 128      # 16 s-tiles
    SC = S // 512      # 4 512-chunks
    NG = ST // 2       # 8 groups of 256 s_q columns

    with tile.TileContext(nc) as tc:
        with (
            tc.tile_pool(name="persist", bufs=1) as persist,
            tc.tile_pool(name="wstage", bufs=1) as wstage,
            tc.tile_pool(name="hstage", bufs=1) as hstage,
            tc.tile_pool(name="hwork", bufs=1) as hwork,
            tc.tile_pool(name="qstage", bufs=1) as qstage,
            tc.tile_pool(name="ework", bufs=4) as ework,
            tc.tile_pool(name="eqwork", bufs=3) as eqwork,
            tc.tile_pool(name="pqu", bufs=3) as pqu_pool,
            tc.tile_pool(name="pqt", bufs=4) as pqt_pool,
            tc.tile_pool(name="xwork", bufs=3) as xwork,
            tc.tile_pool(name="outst", bufs=3) as outst,
            tc.tile_pool(name="small", bufs=24) as small,
            tc.tile_pool(name="ps_mm", bufs=2, space="PSUM") as ps_mm,
            tc.tile_pool(name="ps_s", bufs=3, space="PSUM") as ps_s,
        ):
            # ---- weight load + quantization (small; DVE) ----
            def quant_weight(dram, rs, shape, tag):
                kt = shape[0] // 128
                st_ = wstage.tile([128, kt, shape[1]], F32, tag="wst")
                nc.sync.dma_start(
                    out=st_[:], in_=dram.rearrange("(t p) e -> p t e", p=128)
                )
                tmp = wstage.tile([128, kt, shape[1]], F32, tag="wtmp")
                if wclamp:
                    nc.vector.tensor_scalar(
                        out=tmp[:], in0=st_[:], scalar1=rs, scalar2=127.0,
                        op0=OP.mult, op1=OP.min,
                    )
                    nc.vector.tensor_scalar(
                        out=tmp[:], in0=tmp[:], scalar1=-128.0, scalar2=MAGIC,
                        op0=OP.max, op1=OP.add,
                    )
                else:
                    nc.vector.tensor_scalar(
                        out=tmp[:], in0=st_[:], scalar1=rs, scalar2=MAGIC,
                        op0=OP.mult, op1=OP.add,
                    )
                wi = persist.tile([128, kt, shape[1]], BF16, tag=tag)
                nc.vector.tensor_scalar(
                    out=wi[:], in0=tmp[:], scalar1=MAGIC, scalar2=None,
                    op0=OP.subtract,
                )
                return wi

            wq_i = quant_weight(wqT, rswq, (D, EPC), "wq_i")
            wk_i = quant_weight(wkT, rswk, (D, EPC), "wk_i")
            wv_i = quant_weight(wvT, rswv, (D, EPC), "wv_i")
            wo_i = quant_weight(woT, rswo, (EPC, D), "wo_i")

            # ---- hs load + quantization (DVE 3-pass; needs the clamp) ----
            hsq = persist.tile([128, DT, S], BF16, tag="hsq")
            for t in range(DT):
                hst = hstage.tile([128, S], F32, tag="hst")
                nc.sync.dma_start(
                    out=hst[:],
                    in_=hsT.rearrange("(t p) s -> p t s", p=128)[:, t, :],
                )
                t1 = hwork.tile([128, S], F32, tag="hq1")
                nc.vector.tensor_scalar(
                    out=t1[:], in0=hst[:], scalar1=rs0, scalar2=127.0,
                    op0=OP.mult, op1=OP.min,
                )
                nc.vector.tensor_scalar(
                    out=t1[:], in0=t1[:], scalar1=-128.0, scalar2=MAGIC,
                    op0=OP.max, op1=OP.add,
                )
                nc.vector.tensor_scalar(
                    out=hsq[:, t, :], in0=t1[:], scalar1=MAGIC, scalar2=None,
                    op0=OP.subtract,
                )

            # ---- q/k/v projections (2-pass quant, no clamp: ~6 sigma) ----
            qT_b = persist.tile([128, ET, S], BF16, tag="qT_b")
            kT_b = persist.tile([128, ET, S], BF16, tag="kT_b")
            v_b = persist.tile([128, ST, EPC], BF16, tag="v_b")

            def proj_group(cs, dst_slice, n_chunks, chunk_w, lhsT_fn, rhs_fn):
                """n_chunks matmul chunks -> f32 staging; one GPSIMD pass
                finishes rounding into fp16 dst (whole group)."""
                stage = qstage.tile([128, n_chunks * chunk_w], F32, tag="qst")
                for ci in range(n_chunks):
                    pq = ps_mm.tile([128, 512], F32, tag="mm")
                    for kt in range(DT):
                        nc.tensor.matmul(
                            pq[:, :chunk_w], lhsT_fn(ci, kt), rhs_fn(ci, kt),
                            start=(kt == 0), stop=(kt == DT - 1),
                        )
                    # DVE PSUM evac fused with the magic add (single rounding)
                    nc.vector.tensor_scalar(
                        out=stage[:, ci * chunk_w:(ci + 1) * chunk_w],
                        in0=pq[:, :chunk_w], scalar1=cs, scalar2=MAGIC,
                        op0=OP.mult, op1=OP.add,
                    )
                raw = qstage.tile([128, n_chunks * chunk_w], BF16, tag="qraw")
                nc.gpsimd.tensor_scalar(
                    out=raw[:], in0=stage[:], scalar1=1.0, scalar2=-MAGIC,
                    op0=OP.mult, op1=OP.add,
                )
                # round-then-clamp == reference clamp-then-round (int rails);
                # fp16->fp16 runs in the DVE 4x path
                nc.vector.tensor_scalar(
                    out=dst_slice, in0=raw[:], scalar1=127.0, scalar2=-128.0,
                    op0=OP.min, op1=OP.max,
                )

            for mt in range(ET):
                for wi, cs, dst in ((wq_i, cq, qT_b), (wk_i, ck, kT_b)):
                    proj_group(
                        cs, dst[:, mt, :], SC, 512,
                        lambda ci, kt, wi=wi, mt=mt: wi[:, kt, mt * 128:(mt + 1) * 128],
                        lambda ci, kt: hsq[:, kt, ci * 512:(ci + 1) * 512],
                    )

            # ---- attention main loop ----
            xTb = persist.tile([128, ET, S], BF16, tag="xTb")

            def do_block(mt, sq, dsts, sqi):
                """Scores + exp + sums + quantize + transpose for heads
                (2mt, 2mt+1) x s_q rows [sq*128, (sq+1)*128)."""
                e_tiles = []
                sums = {0: [], 1: []}
                for which in range(2):
                    ee = ework.tile([128, S], F32, tag="e")
                    e_tiles.append(ee)
                for half in range(2):
                    pssA = ps_s.tile([128, 1024], F32, tag="sc")
                    pssB = ps_s.tile([128, 1024], F32, tag="sc")
                    for ckk in range(2):
                        nn = half * 2 + ckk
                        nc.tensor.matmul(
                            pssA[:, ckk * 512:(ckk + 1) * 512],
                            qT_b[0:64, mt, sq * 128:(sq + 1) * 128],
                            kT_b[0:64, mt, nn * 512:(nn + 1) * 512],
                            start=True, stop=True,
                            tile_position=(0, 0),
                        )
                        nc.tensor.matmul(
                            pssB[:, ckk * 512:(ckk + 1) * 512],
                            qT_b[64:128, mt, sq * 128:(sq + 1) * 128],
                            kT_b[64:128, mt, nn * 512:(nn + 1) * 512],
                            start=True, stop=True,
                            tile_position=(64, 0),
                        )
                    for which, pss in ((0, pssA), (1, pssB)):
                        sh = small.tile([128, 1], F32, tag="sh")
                        nc.scalar.activation(
                            out=e_tiles[which][:, half * 1024:(half + 1) * 1024],
                            in_=pss[:], func=AF.Exp,
                            bias=0.0, scale=ce, accum_out=sh[:],
                        )
                        sums[which].append(sh)
                out = []
                r255s = []
                for which in range(2):
                    ss = small.tile([128, 1], F32, tag="ss")
                    nc.vector.tensor_add(ss[:], sums[which][0][:], sums[which][1][:])
                    s255 = small.tile([128, 1], F32, tag="s255")
                    nc.vector.tensor_scalar(
                        out=s255[:], in0=ss[:],
                        scalar1=float(np.float32(1.0 / (255.0 * 8192.0))),
                        scalar2=None, op0=OP.mult,
                    )
                    r255 = small.tile([128, 1], F32, tag="r255")
                    nc.vector.reciprocal(r255[:], s255[:])
                    r255s.append(r255)
                for which in range(2):
                    r255 = r255s[which]
                    # pass 1 (GPSIMD): e*(8192*255/sum) + 1.5*2^36
                    #   -> f32 rounded to 8192*round(255*e/sum)  (single RNE)
                    eq = eqwork.tile([128, S], F32, tag="eq")
                    nc.gpsimd.tensor_scalar(
                        out=eq[:], in0=e_tiles[which][:],
                        scalar1=r255[:], scalar2=M8,
                        op0=OP.mult, op1=OP.add,
                    )
                    # pass 2: *2^-13, +(1536 - 1.5*2^23) -> fp16 = pq + 1536
                    pq_t = pqu_pool.tile([128, S], BF16, tag="pqu")
                    # odd split keeps DVE in 1-port mode (no GPSIMD lockout)
                    for lo, hi in ((0, 1023), (1023, S)):
                        nc.vector.tensor_scalar(
                            out=pq_t[:, lo:hi], in0=eq[:, lo:hi],
                            scalar1=INV8K, scalar2=-MAGIC,
                            op0=OP.mult, op1=OP.add,
                        )
                    nc.sync.dma_start(
                        out=dsts[which][:, sqi, :, :], in_=pq_t[:],
                        transpose=True,
                    )
                    out.append(pq_t)
                return out

            def transpose_block(pq_t, dst, sqi):
                # dst: [128, 2, ST, 128] fp16; sub-dst contiguous 4KB/partition
                nc.sync.dma_start(
                    out=dst[:, sqi, :, :], in_=pq_t[:], transpose=True,
                )

            def emit_pv_pair(hA, g, ptb):
                # heads hA (psum rows 0:64) and hA+1 (rows 64:128), col-packed
                mt = hA // 2
                po = ps_mm.tile([128, 512], F32, tag="mm")  # use cols 0:256
                for tt in range(ST):
                    first = (tt == 0)
                    last = (tt == ST - 1)
                    # rhs [128, 2, 128] streams both s_q half-groups (N=256)
                    nc.tensor.matmul(
                        po[0:64, 0:256],
                        v_b[:, tt, 64 * hA:64 * hA + 64],
                        ptb[0][:, :, tt, :],
                        start=first, stop=last,
                        tile_position=(0, 0), skip_group_check=True,
                    )
                    nc.tensor.matmul(
                        po[64:128, 0:256],
                        v_b[:, tt, 64 * hA + 64:64 * hA + 128],
                        ptb[1][:, :, tt, :],
                        start=first, stop=last,
                        tile_position=(0, 64), skip_group_check=True,
                    )
                # evac: x = po*cx - 1536*colsum(v)*cx, then clamp
                xt = xwork.tile([128, 256], F32, tag="xt")
                nc.vector.tensor_scalar(
                    out=xt[:], in0=po[:, 0:256], scalar1=cx, scalar2=127.0,
                    op0=OP.mult, op1=OP.min,
                )
                t2 = xwork.tile([128, 256], F32, tag="xc")
                nc.vector.tensor_scalar(
                    out=t2[:], in0=xt[:], scalar1=-128.0, scalar2=MAGIC,
                    op0=OP.max, op1=OP.add,
                )
                nc.vector.tensor_scalar(
                    out=xTb[:, mt, g * 256:(g + 1) * 256], in0=t2[:],
                    scalar1=MAGIC, scalar2=None, op0=OP.subtract,
                )

            def emit_outproj_chunk(c):
                # output projection for s columns [c*512, (c+1)*512)
                for mtd in range(DT):
                    pf = ps_mm.tile([128, 512], F32, tag="mm")
                    for kt in range(ET):
                        nc.tensor.matmul(
                            pf[:],
                            wo_i[:, kt, mtd * 128:(mtd + 1) * 128],
                            xTb[:, kt, c * 512:(c + 1) * 512],
                            start=(kt == 0), stop=(kt == ET - 1),
                        )
                    ot = outst.tile([128, 512], F32, tag="ot")
                    nc.vector.tensor_scalar(
                        out=ot[:], in0=pf[:], scalar1=cout, scalar2=None,
                        op0=OP.mult,
                    )
                    nc.sync.dma_start(
                        out=outT[mtd * 128:(mtd + 1) * 128,
                                 c * 512:(c + 1) * 512],
                        in_=ot[:],
                    )

            for g in range(NG):
                for mt in range(ET):
                    ptbs = []
                    for h in (2 * mt, 2 * mt + 1):
                        ptbs.append(pqt_pool.tile(
                            [128, 2, ST, 128], BF16, tag="ptb", name="ptb"
                        ))
                    for sqi in range(2):
                        sq = 2 * g + sqi
                        do_block(mt, sq, ptbs, sqi)
                        if g == 0 and mt == 0 and sqi == 0:
                            for sg in range(4):
                                proj_group(
                                    cv, v_b[:, sg * 4:(sg + 1) * 4, :], 4, EPC,
                                    lambda ci, kt, sg=sg: hsq[:, kt, (sg * 4 + ci) * 128:(sg * 4 + ci + 1) * 128],
                                    lambda ci, kt: wv_i[:, kt, :],
                                )
                    emit_pv_pair(2 * mt, g, ptbs)
                if g % 2 == 1:
                    emit_outproj_chunk((g - 1) // 2)

    nc.finalize()
    return nc


def kernel(hs, Wq, Wk, Wv, Wo, bo, scales, **_ignored):
    hs = np.asarray(hs, dtype=np.float32)
    Wq = np.asarray(Wq, dtype=np.float32)
    Wk = np.asarray(Wk, dtype=np.float32)
    Wv = np.asarray(Wv, dtype=np.float32)
    Wo = np.asarray(Wo, dtype=np.float32)
    bo = np.asarray(bo, dtype=np.float32)
    sc = np.asarray(scales, dtype=np.float32)

    one = np.float32(1.0)
    # The reference requantizes hs by s1/s3/s5 after quantizing by s0; with
    # s1 == s3 == s5 == s0 (as set up) that is an exact no-op on the integers.
    assert np.allclose(sc[1], sc[0]) and np.allclose(sc[3], sc[0]) and np.allclose(sc[5], sc[0])
    assert np.allclose(sc[9], one / np.float32(255.0)) and np.allclose(sc[10], sc[9])

    consts = (
        float(one / sc[0]),                       # rs0
        float(one / sc[2]),                       # rswq
        float(one / sc[4]),                       # rswk
        float(one / sc[6]),                       # rswv
        float(one / sc[13]),                      # rswo
        float(sc[1] * sc[2] / sc[7]),             # cq
        float(sc[3] * sc[4] / sc[8]),             # ck
        float(sc[5] * sc[6] / sc[11]),            # cv
        float(sc[7] * sc[8] * np.float32(DH ** -0.5)),  # ce
        bool(max(
            float(np.abs(Wq).max() / sc[2]), float(np.abs(Wk).max() / sc[4]),
            float(np.abs(Wv).max() / sc[6]), float(np.abs(Wo).max() / sc[13]),
        ) > 126.49),                              # wclamp needed?
        float(sc[11] / np.float32(255.0) / sc[12]),     # cx
        float(sc[12] * sc[13]),                   # cout
    )

    if consts not in _prog_cache:
        _prog_cache[consts] = _build(consts)
    nc = _prog_cache[consts]

    in_maps = []
    for c in range(N_CORES):
        b = c // 2
        g = c % 2
        es = slice(g * EPC, (g + 1) * EPC)
        in_maps.append({
            "hsT": np.ascontiguousarray(hs[b].T),
            "wqT": np.ascontiguousarray(Wq.T[:, es]),
            "wkT": np.ascontiguousarray(Wk.T[:, es]),
            "wvT": np.ascontiguousarray(Wv.T[:, es]),
            "woT": np.ascontiguousarray(Wo.T[es, :]),
        })

    res = run_bass_kernel_spmd(nc, in_maps, list(range(N_CORES)))
    outs = res.results

    out = np.empty((B, S, D), dtype=np.float32)
    for b in range(B):
        acc = outs[2 * b]["outT"] + outs[2 * b + 1]["outT"]
        out[b] = acc.T + bo[None, :]
    return out
